# revision 15
# baseline (speedup 1.0000x reference)
"""Trainium2 Bass kernel for nn_DistLayer (GNN message passing layer).

Computes, for full inputs (see reference):
    pa = relu(seg_mean(x[:, :128], atom_idx, 1024))[atom_idx]
    pe = relu(seg_mean(x[:, 128:], ele_idx, 100))[ele_idx]
    h  = concat([dist_feat, pa, pe], 1) @ W1 (+ b1)
    out = relu(batchnorm_train(h; gamma, beta) + x)

Note b1 provably cancels in (h - mean(h)), so it is ignored.

v6 design (8 cores, data-parallel over rows):
  - Rows sharded 25000/core; atom segments are packed into 8 windows of
    128 segments by a balanced greedy assignment (minimizes the max
    per-core window load), so each window needs only 3200 padded rows.
  - DMA is descriptor-rate-bound (~17ns/descriptor/queue), so every bulk
    DRAM layout is partition-blocked with >=6.1KB contiguous runs, and
    bulk traffic is split across the two hardware DGE rings (sync=SP,
    scalar=Activation).  GpSimd (software DGE) carries only the tiny
    collective bounce buffers.
  - Stage A (segment sums): fp8 row-major one-hots (atom|ele interleaved
    per chunk) against bf16 x chunks; ele accumulates in one PSUM chain
    across all windows.
  - AllReduce #1 is split (windows 0-3 early at w3, 4-7 + ele at stage A
    end) and a tiny warm-up AllReduce absorbs the first-collective cost.
  - Stage C computes h TRANSPOSED ([feat, rows]): lhsT = W1 / tables,
    rhs = dsT and fp8 transposed one-hots (atom|ele interleaved per
    512-col group).  BN sums fuse into the PSUM->SBUF copy via
    activation(Copy, accum_out=...); Sum(h^2) via one DVE
    scalar_tensor_tensor per group half.
  - AllReduce #2 carries [128, 4] per-feature sums; BN scale/shift
    become per-partition scalars, so stage E is one DVE op
    (h*A + xT, written in place over the xT tile) + one scalar-engine
    relu(.. + B), reading a transposed x copy and writing a transposed
    output (host undoes the layout).
"""
import sys

sys.path.insert(0, "/opt/trn_rl_repo")

import numpy as np

import concourse.bass as bass
import concourse.mybir as mybir
import concourse.tile as tile
from concourse import bacc
from concourse.bass_utils import run_bass_kernel_spmd, axon_active

# problem constants
N = 200000
NAE = 128
NDE = 128
G = 1024
E = 100
NCORES = 8
RPC = N // NCORES          # 25000 rows per core
NW = 8                     # atom segment windows of 128
CPW = 25                   # chunks (of 128 rows) per window
BUCKET = CPW * 128         # 3200 padded rows per window
TROWS = NW * BUCKET        # 25600 padded rows per core
T = TROWS // 128           # 200 chunks
SUMW = G + 128             # 1152: [atom sums | ele sums(padded to 128)]
EPS = 1e-5
INV_N = 1.0 / N
EGROUP = 3200              # stage-E columns per group (8 groups)

F32 = mybir.dt.float32
BF16 = mybir.dt.bfloat16
FP8 = mybir.dt.float8e4

_CACHED_PROGRAM = None


def _build_program(debug=None, probe=False):
    dbg = (not axon_active()) if debug is None else debug
    nc = bacc.Bacc(
        "TRN2",
        target_bir_lowering=False,
        debug=dbg,
        num_devices=NCORES,
    )

    # per-core external I/O (bulk tensors partition-blocked, bf16/fp8)
    x_blk = nc.dram_tensor("x_blk", [128, T * 256], BF16, kind="ExternalInput")
    ohb = nc.dram_tensor("ohb", [128, T * 256], FP8, kind="ExternalInput")
    dsTb = nc.dram_tensor("dsTb", [NDE, TROWS], BF16, kind="ExternalInput")
    ohc = nc.dram_tensor("ohc", [128, 2 * TROWS], FP8, kind="ExternalInput")
    xT = nc.dram_tensor("xT", [128, 2 * TROWS], BF16, kind="ExternalInput")
    w1 = nc.dram_tensor("w1", [3 * 128, 2 * NAE], F32, kind="ExternalInput")
    gbc = nc.dram_tensor("gbc", [128, 4], F32, kind="ExternalInput")
    rcb = nc.dram_tensor("rcb", [128, SUMW], F32, kind="ExternalInput")
    out_d = nc.dram_tensor("out", [128, 2 * TROWS], BF16, kind="ExternalOutput")
    if probe:
        dbg_acc = nc.dram_tensor("dbg_acc", [128, SUMW], F32, kind="ExternalOutput")
        dbg_rm = nc.dram_tensor("dbg_rm", [128, SUMW], BF16, kind="ExternalOutput")
        dbg_s = nc.dram_tensor("dbg_s", [128, 4 * 56], F32, kind="ExternalOutput")
        dbg_pre = nc.dram_tensor("dbg_pre", [128, 4], F32, kind="ExternalOutput")
        dbg_post = nc.dram_tensor("dbg_post", [128, 4], F32, kind="ExternalOutput")
        dbg_ab = nc.dram_tensor("dbg_ab", [128, 10], F32, kind="ExternalOutput")
        dbg_h = nc.dram_tensor("dbg_h", [128, 4096], BF16, kind="ExternalOutput")

    # internal DRAM (collective bounce buffers)
    ccw_in = nc.dram_tensor("ccw_in", [1, 16], BF16)
    ccw_out = nc.dram_tensor("ccw_out", [1, 16], BF16, addr_space="Shared")
    cc1a_in = nc.dram_tensor("cc1a_in", [128, 512], BF16)
    cc1a_out = nc.dram_tensor("cc1a_out", [128, 512], BF16, addr_space="Shared")
    cc1b_in = nc.dram_tensor("cc1b_in", [128, SUMW - 512], BF16)
    cc1b_out = nc.dram_tensor("cc1b_out", [128, SUMW - 512], BF16, addr_space="Shared")
    cc2_in = nc.dram_tensor("cc2_in", [128, 4], F32)
    cc2_out = nc.dram_tensor("cc2_out", [128, 4], F32, addr_space="Shared")

    RELU = mybir.ActivationFunctionType.Relu
    COPY = mybir.ActivationFunctionType.Copy
    SQRT = mybir.ActivationFunctionType.Sqrt
    MULT = mybir.AluOpType.mult
    ADD = mybir.AluOpType.add
    AX = mybir.AxisListType.X

    NGW = (BUCKET + 511) // 512        # stage-C col groups per window (7)
    NGROUPS = NW * NGW                 # 56
    NEG = TROWS // EGROUP              # stage-E col groups (8)

    def rgrp():
        return [list(range(NCORES))]

    with tile.TileContext(nc) as tc:
        with (
            tc.tile_pool(name="const", bufs=1) as cp,
            tc.tile_pool(name="hcache", bufs=1) as hp,
            tc.tile_pool(name="xload", bufs=2) as xp,
            tc.tile_pool(name="cload", bufs=2) as dp,
            tc.tile_pool(name="work", bufs=2) as wp,
            tc.tile_pool(name="outp", bufs=2) as op_,
        ):
            # ---- warm-up collective (absorbs first-op/barrier cost)
            wu = cp.tile([1, 16], BF16, tag="wu")
            nc.vector.memset(wu[:], 0.0)
            nc.gpsimd.dma_start(ccw_in[:], wu[:])
            nc.gpsimd.collective_compute(
                "AllReduce", ADD, replica_groups=rgrp(),
                ins=[ccw_in[:]], outs=[ccw_out[:]],
            )

            # ---- constants into SBUF (scalar ring)
            w1bf = []
            for i in range(3):
                tf = wp.tile([128, 256], F32, tag="w1f", bufs=1)
                nc.scalar.dma_start(tf[:], w1[i * 128 : (i + 1) * 128, :])
                tb = cp.tile([128, 256], BF16, tag=f"w1b{i}")
                nc.scalar.copy(tb[:], tf[:])
                w1bf.append(tb)
            w1d, w1a, w1e = w1bf

            rcb_sb = cp.tile([128, SUMW], F32, tag="rcb")
            nc.scalar.dma_start(rcb_sb[:], rcb[:])
            gbc_sb = cp.tile([128, 4], F32, tag="gbc")
            nc.scalar.dma_start(gbc_sb[:], gbc[:])

            # ---- Stage A: local segment sums acc[feat, seg]
            acc = cp.tile([128, SUMW], F32, tag="acc")
            accb = cp.tile([128, SUMW], BF16, tag="accb")

            psA = tc.alloc_tile_pool(name="psA", bufs=2, space="PSUM")
            psE = tc.alloc_tile_pool(name="psE", bufs=1, space="PSUM")
            ps_e = psE.tile([128, 128], F32, tag="ps_e")
            for w in range(NW):
                ps_a = psA.tile([128, 128], F32, tag="ps_a")
                oh = dp.tile([128, CPW * 256], FP8, tag="ohb")
                nc.scalar.dma_start(
                    oh[:], ohb[:, w * CPW * 256 : (w + 1) * CPW * 256]
                )
                for b0, nb in ((0, 13), (13, 12)):
                    t0 = w * CPW + b0
                    xq = xp.tile([128, 13 * 256], BF16, tag="xq")
                    nc.sync.dma_start(
                        xq[:, 0 : nb * 256],
                        x_blk[:, t0 * 256 : (t0 + nb) * 256],
                    )
                    for j in range(nb):
                        t = t0 + j
                        jj = b0 + j
                        nc.tensor.matmul(
                            ps_a[:],
                            lhsT=xq[:, j * 256 : j * 256 + 128],
                            rhs=oh[:, jj * 256 : jj * 256 + 128],
                            start=(jj == 0),
                            stop=(jj == CPW - 1),
                        )
                        nc.tensor.matmul(
                            ps_e[:],
                            lhsT=xq[:, j * 256 + 128 : (j + 1) * 256],
                            rhs=oh[:, jj * 256 + 128 : (jj + 1) * 256],
                            start=(t == 0),
                            stop=(t == T - 1),
                        )
                nc.vector.tensor_copy(acc[:, w * 128 : (w + 1) * 128], ps_a[:])
                if w == 3:
                    # early AllReduce for windows 0-3 while 4-7 compute
                    nc.vector.tensor_copy(accb[:, 0:512], acc[:, 0:512])
                    nc.gpsimd.dma_start(cc1a_in[:], accb[:, 0:512])
                    nc.gpsimd.collective_compute(
                        "AllReduce", ADD, replica_groups=rgrp(),
                        ins=[cc1a_in[:]], outs=[cc1a_out[:]],
                    )
            nc.vector.tensor_copy(acc[:, G : G + 128], ps_e[:])
            psE.release()
            psA.release()

            # ---- AllReduce #1 tail (windows 4-7 + ele sums)
            nc.vector.tensor_copy(accb[:, 512:SUMW], acc[:, 512:SUMW])
            nc.gpsimd.dma_start(cc1b_in[:], accb[:, 512:SUMW])
            nc.gpsimd.collective_compute(
                "AllReduce", ADD, replica_groups=rgrp(),
                ins=[cc1b_in[:]], outs=[cc1b_out[:]],
            )
            if probe:
                nc.gpsimd.dma_start(dbg_acc[:], acc[:])

            # ---- tables: tbl[s, feat2] = relu(mean)[s,:] @ W1 part
            rm = cp.tile([128, SUMW], BF16, tag="rm")
            tbl_a = cp.tile([128, NW * 256], BF16, tag="tbl_a")
            tbl_e = cp.tile([128, 256], BF16, tag="tbl_e")

            psT = tc.alloc_tile_pool(name="psT", bufs=2, space="PSUM")
            psC = tc.alloc_tile_pool(name="psC", bufs=2, space="PSUM")

            def tables_for(lo, hi, windows, with_ele):
                # acc is dead after the accb converts; reuse it as scratch
                nc.gpsimd.dma_start(
                    accb[:, lo:hi], (cc1a_out if lo == 0 else cc1b_out)[:]
                )
                nc.vector.tensor_mul(
                    acc[:, lo:hi], accb[:, lo:hi], rcb_sb[:, lo:hi]
                )
                nc.vector.tensor_scalar_max(rm[:, lo:hi], acc[:, lo:hi], 0.0)
                for w in windows:
                    pst = psT.tile([128, 256], F32, tag="pst")
                    nc.tensor.matmul(
                        pst[:],
                        lhsT=rm[:, w * 128 : (w + 1) * 128],
                        rhs=w1a[:],
                        start=True, stop=True,
                    )
                    nc.scalar.copy(tbl_a[:, w * 256 : (w + 1) * 256], pst[:])
                if with_ele:
                    pst = psT.tile([128, 256], F32, tag="pst")
                    nc.tensor.matmul(
                        pst[:], lhsT=rm[:, G : G + 128], rhs=w1e[:],
                        start=True, stop=True,
                    )
                    nc.scalar.copy(tbl_e[:], pst[:])

            # ---- Stage C: hT[feat, rows] = W1d.T@dsT + tblA.T@ohtA + tblE.T@ohtE
            hT0 = hp.tile([128, TROWS], BF16, tag="hT0")
            hT1 = hp.tile([128, TROWS], BF16, tag="hT1")
            s1a = cp.tile([128, NGROUPS], F32, tag="s1a")
            s1b = cp.tile([128, NGROUPS], F32, tag="s1b")
            s2a = cp.tile([128, NGROUPS], F32, tag="s2a")
            s2b = cp.tile([128, NGROUPS], F32, tag="s2b")

            def stagec_window(w):
                base = w * BUCKET
                dq = dp.tile([128, BUCKET], BF16, tag="dq")
                nc.sync.dma_start(dq[:], dsTb[:, base : base + BUCKET])
                oc = dp.tile([128, 2 * BUCKET], FP8, tag="ohc")
                nc.scalar.dma_start(
                    oc[:], ohc[:, w * 2 * BUCKET : (w + 1) * 2 * BUCKET]
                )
                for g in range(NGW):
                    g0 = g * 512
                    gl = min(512, BUCKET - g0)
                    gi = w * NGW + g
                    oco = g * 1024
                    psL = psC.tile([128, 512], F32, tag="psL")
                    psH = psC.tile([128, 512], F32, tag="psH")
                    for half, pst in ((0, psL), (1, psH)):
                        o = half * 128
                        nc.tensor.matmul(
                            pst[:, 0:gl], lhsT=w1d[:, o : o + 128],
                            rhs=dq[:, g0 : g0 + gl], start=True, stop=False,
                        )
                        nc.tensor.matmul(
                            pst[:, 0:gl],
                            lhsT=tbl_a[:, w * 256 + o : w * 256 + o + 128],
                            rhs=oc[:, oco : oco + gl], start=False, stop=False,
                        )
                        nc.tensor.matmul(
                            pst[:, 0:gl], lhsT=tbl_e[:, o : o + 128],
                            rhs=oc[:, oco + gl : oco + 2 * gl],
                            start=False, stop=True,
                        )
                    nc.scalar.activation(
                        hT0[:, base + g0 : base + g0 + gl], psL[:, 0:gl],
                        COPY, accum_out=s1a[:, gi : gi + 1],
                    )
                    nc.scalar.activation(
                        hT1[:, base + g0 : base + g0 + gl], psH[:, 0:gl],
                        COPY, accum_out=s1b[:, gi : gi + 1],
                    )
                    h0s = hT0[:, base + g0 : base + g0 + gl]
                    h1s = hT1[:, base + g0 : base + g0 + gl]
                    sq = wp.tile([128, 512], BF16, tag="sq", bufs=2)
                    nc.vector.scalar_tensor_tensor(
                        sq[:, 0:gl], h0s, 1.0, h0s,
                        MULT, MULT, accum_out=s2a[:, gi : gi + 1],
                    )
                    sq2 = wp.tile([128, 512], BF16, tag="sq", bufs=2)
                    nc.vector.scalar_tensor_tensor(
                        sq2[:, 0:gl], h1s, 1.0, h1s,
                        MULT, MULT, accum_out=s2b[:, gi : gi + 1],
                    )

            tables_for(0, 512, range(4), False)
            tables_for(512, SUMW, range(4, 8), True)
            for w in range(NW):
                stagec_window(w)

            # ---- BN stats: reduce, AllReduce #2, per-partition A/B
            sstat = cp.tile([128, 4], F32, tag="sstat")
            nc.vector.tensor_reduce(sstat[:, 0:1], s1a[:], AX, ADD)
            nc.vector.tensor_reduce(sstat[:, 1:2], s1b[:], AX, ADD)
            nc.vector.tensor_reduce(sstat[:, 2:3], s2a[:], AX, ADD)
            nc.vector.tensor_reduce(sstat[:, 3:4], s2b[:], AX, ADD)
            if probe:
                nc.gpsimd.dma_start(dbg_pre[:], sstat[:])
                nc.gpsimd.dma_start(dbg_s[:, 0:56], s1a[:])
                nc.gpsimd.dma_start(dbg_s[:, 56:112], s1b[:])
                nc.gpsimd.dma_start(dbg_s[:, 112:168], s2a[:])
                nc.gpsimd.dma_start(dbg_s[:, 168:224], s2b[:])
            nc.gpsimd.dma_start(cc2_in[:], sstat[:])
            nc.gpsimd.collective_compute(
                "AllReduce", ADD, replica_groups=rgrp(),
                ins=[cc2_in[:]], outs=[cc2_out[:]],
            )
            nc.gpsimd.dma_start(sstat[:], cc2_out[:])

            mu = cp.tile([128, 2], F32, tag="mu")
            nc.vector.tensor_scalar_mul(mu[:], sstat[:, 0:2], INV_N)
            ex2 = cp.tile([128, 2], F32, tag="ex2")
            nc.vector.tensor_scalar_mul(ex2[:], sstat[:, 2:4], INV_N)
            mu2 = cp.tile([128, 2], F32, tag="mu2")
            nc.vector.tensor_mul(mu2[:], mu[:], mu[:])
            var = cp.tile([128, 2], F32, tag="var")
            nc.vector.tensor_sub(var[:], ex2[:], mu2[:])
            veps = cp.tile([128, 1], F32, tag="veps")
            nc.vector.memset(veps[:], EPS)
            std = cp.tile([128, 2], F32, tag="std")
            nc.scalar.activation(std[:], var[:], SQRT, bias=veps[:])
            rstd = cp.tile([128, 2], F32, tag="rstd")
            nc.vector.reciprocal(rstd[:], std[:])
            Ab = cp.tile([128, 2], F32, tag="Ab")
            nc.vector.tensor_mul(Ab[:], rstd[:], gbc_sb[:, 0:2])
            mA = cp.tile([128, 2], F32, tag="mA")
            nc.vector.tensor_mul(mA[:], mu[:], Ab[:])
            Bb = cp.tile([128, 2], F32, tag="Bb")
            nc.vector.tensor_sub(Bb[:], gbc_sb[:, 2:4], mA[:])
            if probe:
                nc.gpsimd.dma_start(dbg_post[:], sstat[:])
                nc.gpsimd.dma_start(dbg_ab[:, 0:2], mu[:])
                nc.gpsimd.dma_start(dbg_ab[:, 2:4], var[:])
                nc.gpsimd.dma_start(dbg_ab[:, 4:6], rstd[:])
                nc.gpsimd.dma_start(dbg_ab[:, 6:8], Ab[:])
                nc.gpsimd.dma_start(dbg_ab[:, 8:10], Bb[:])
                nc.gpsimd.dma_start(dbg_rm[:], rm[:])
                nc.gpsimd.dma_start(dbg_h[:, 0:2048], hT0[:, 0:2048])
                nc.gpsimd.dma_start(dbg_h[:, 2048:4096], hT1[:, 0:2048])

            # ---- Stage E: outT = relu(hT * A + B + xT), in place over xT tiles
            for o in range(NEG):
                c0 = o * EGROUP
                for half, ht in ((0, hT0), (1, hT1)):
                    off = half * TROWS + c0
                    xe = xp.tile([128, EGROUP], BF16, tag="xe", bufs=3)
                    nc.sync.dma_start(xe[:], xT[:, off : off + EGROUP])
                    nc.vector.scalar_tensor_tensor(
                        xe[:], ht[:, c0 : c0 + EGROUP],
                        Ab[:, half : half + 1], xe[:], MULT, ADD,
                    )
                    ot = op_.tile([128, EGROUP], BF16, tag="ot")
                    nc.scalar.activation(
                        ot[:], xe[:], RELU, bias=Bb[:, half : half + 1]
                    )
                    nc.scalar.dma_start(out_d[:, off : off + EGROUP], ot[:])

            psC.release()
            psT.release()

    nc.compile()
    return nc


def _get_program():
    global _CACHED_PROGRAM
    if _CACHED_PROGRAM is None:
        _CACHED_PROGRAM = _build_program()
    return _CACHED_PROGRAM


def _balance_windows(atom_idx):
    """Assign each of G segments to one of NW windows (128 each),
    minimizing the max per-(core,window) row count."""
    c = np.zeros((G, NCORES), np.int64)
    for k in range(NCORES):
        c[:, k] = np.bincount(atom_idx[k * RPC : (k + 1) * RPC], minlength=G)
    order = np.argsort(-c.sum(1), kind="stable")
    load = np.zeros((NW, NCORES), np.int64)
    cnt = np.zeros(NW, np.int64)
    win = np.zeros(G, np.int64)
    slot = np.zeros(G, np.int64)
    for g in order:
        best, bestv = -1, None
        for w in range(NW):
            if cnt[w] >= 128:
                continue
            key = (int((load[w] + c[g]).max()), int(load[w].sum()))
            if best < 0 or key < bestv:
                best, bestv = w, key
        win[g] = best
        slot[g] = cnt[best]
        load[best] += c[g]
        cnt[best] += 1
    if load.max() > BUCKET:
        raise RuntimeError(f"window overflow: {load.max()} > {BUCKET}")
    return win, slot


def _prepare(x, dist_feat, atom_idx, ele_idx, W1, gamma, beta):
    """Shard+plan all cores; returns (in_maps, positions)."""
    import ml_dtypes

    BF = ml_dtypes.bfloat16
    F8 = ml_dtypes.float8_e4m3

    x = np.ascontiguousarray(np.asarray(x, dtype=np.float32))
    dist_feat = np.ascontiguousarray(np.asarray(dist_feat, dtype=np.float32))
    atom_idx = np.asarray(atom_idx).astype(np.int64)
    ele_idx = np.asarray(ele_idx).astype(np.int64)
    W1 = np.ascontiguousarray(np.asarray(W1, dtype=np.float32))
    gamma = np.asarray(gamma, dtype=np.float32)
    beta = np.asarray(beta, dtype=np.float32)

    win, slot = _balance_windows(atom_idx)

    # reciprocal counts in permuted (window,slot) order
    rc = np.zeros((SUMW,), np.float32)
    gcnt = np.maximum(np.bincount(atom_idx, minlength=G), 1.0)
    rc[win * 128 + slot] = 1.0 / gcnt
    rc[G : G + E] = 1.0 / np.maximum(np.bincount(ele_idx, minlength=E), 1.0)
    rcb = np.ascontiguousarray(np.broadcast_to(rc, (128, SUMW))).astype(np.float32)
    gbc = np.stack(
        [gamma[:128], gamma[128:], beta[:128], beta[128:]], axis=1
    ).astype(np.float32)
    gbc = np.ascontiguousarray(gbc)

    ar128 = np.arange(128, dtype=np.int64)
    NGW = (BUCKET + 511) // 512
    in_maps = []
    positions = []
    for cidx in range(NCORES):
        sl = slice(cidx * RPC, (cidx + 1) * RPC)
        x_s, d_s = x[sl], dist_feat[sl]
        a_s, e_s = atom_idx[sl], ele_idx[sl]

        bucket = win[a_s]
        order = np.argsort(bucket, kind="stable")
        counts = np.bincount(bucket, minlength=NW)
        if counts.max() > BUCKET:
            raise RuntimeError(f"window overflow: {counts.max()} > {BUCKET}")

        xp_ = np.zeros((TROWS, 2 * NAE), np.float32)
        dp_ = np.zeros((TROWS, NDE), np.float32)
        awp = np.full(TROWS, -1.0, np.float32)
        ewp = np.full(TROWS, -1.0, np.float32)
        pos = np.empty(RPC, np.int64)
        start = 0
        for w in range(NW):
            k = counts[w]
            rows = order[start : start + k]
            start += k
            b = w * BUCKET
            xp_[b : b + k] = x_s[rows]
            dp_[b : b + k] = d_s[rows]
            awp[b : b + k] = slot[a_s[rows]]
            ewp[b : b + k] = e_s[rows]
            pos[rows] = np.arange(b, b + k)
        positions.append(pos)

        xb = xp_.astype(BF)
        x_blk = np.ascontiguousarray(
            xb.reshape(T, 128, 256).transpose(1, 0, 2)
        ).reshape(128, T * 256)
        xTb = np.ascontiguousarray(
            xb.T.reshape(2, 128, TROWS).transpose(1, 0, 2)
        ).reshape(128, 2 * TROWS)
        dsTb = np.ascontiguousarray(dp_.T).astype(BF)

        # row-major one-hots, atom|ele interleaved per chunk:
        # ohb[p, t*256 + s] = (awp[t*128+p] == s); [.. + 128 + s] for ele
        a3 = awp.reshape(T, 128)[:, :, None] == ar128[None, None, :]
        e3 = ewp.reshape(T, 128)[:, :, None] == ar128[None, None, :]
        ohb_ = np.ascontiguousarray(
            np.concatenate([a3, e3], axis=2).transpose(1, 0, 2)
        ).reshape(128, T * 256).astype(F8)

        # transposed one-hots, atom|ele interleaved per 512-col group
        ohta_ = awp[None, :] == ar128[:, None]
        ohte_ = ewp[None, :] == ar128[:, None]
        parts = []
        for w in range(NW):
            for g in range(NGW):
                c0 = w * BUCKET + g * 512
                gl = min(512, BUCKET - g * 512)
                parts.append(ohta_[:, c0 : c0 + gl])
                parts.append(ohte_[:, c0 : c0 + gl])
        ohc_ = np.ascontiguousarray(np.concatenate(parts, axis=1)).astype(F8)

        in_maps.append(
            {
                "x_blk": x_blk,
                "ohb": ohb_,
                "dsTb": dsTb,
                "ohc": ohc_,
                "xT": xTb,
                "w1": W1,
                "gbc": gbc,
                "rcb": rcb,
            }
        )
    return in_maps, positions


def kernel(x, dist_feat, atom_idx, ele_idx, W1, b1, gamma, beta, num_graphs, num_eles):
    assert int(num_graphs) == G and int(num_eles) == E
    assert np.asarray(x).shape == (N, 2 * NAE)

    nc = _get_program()
    in_maps, positions = _prepare(x, dist_feat, atom_idx, ele_idx, W1, gamma, beta)
    try:
        res = run_bass_kernel_spmd(nc, in_maps, core_ids=list(range(NCORES)))
    except Exception:
        # transient device errors (rare NRT_EXEC_UNIT_UNRECOVERABLE) - retry once
        res = run_bass_kernel_spmd(nc, in_maps, core_ids=list(range(NCORES)))

    out = np.empty((N, 2 * NAE), np.float32)
    for c in range(NCORES):
        dev = np.asarray(res.results[c]["out"]).astype(np.float32)
        full = dev.reshape(128, 2, TROWS).transpose(2, 1, 0).reshape(TROWS, 256)
        out[c * RPC : (c + 1) * RPC] = full[positions[c]]
    return out
